# revision 1
# baseline (speedup 1.0000x reference)
"""Tensor-parallel DeepSpeed encoder-decoder block on 8 TRN2 NeuronCores.

Sharding (per the mp_group scheme): attn_qkvw / inter_w / inter_w1 are
column-sharded (heads / intermediate dim), attn_ow / output_w row-sharded.
The post-attn_ow all-reduce is implemented as ReduceScatter + AllGather
(same bytes, and the RS output directly provides each core's 128-row
feature stripe of the residual for the final output). The post-output_w
all-reduce is a ReduceScatter only - each core finishes and returns its
own 128-row stripe of out.T, assembled and transposed on the host.

Device layout convention: activations live feature-major (XT = [feat, tok]).
Matmuls are out = W_chunk.T @ XT_chunk accumulated over 128-row chunks in
PSUM (bf16 inputs, f32 accum). LayerNorms are folded into the following
matmul: gamma/beta fold into the weights host-side; the mean subtraction is
a rank-1 (-colsum(W) x mean) correction matmul; the 1/std scaling is a
broadcast multiply at PSUM-drain time. Softmax runs in transposed score
layout (keys on partitions) with the denominator produced by a ones-column
augmentation of V, so no max-subtraction pass and no extra reduction pass.
"""
from contextlib import ExitStack

import numpy as np
import ml_dtypes

import concourse.bacc as bacc
import concourse.mybir as mybir
import concourse.tile as tile
from concourse import masks
from concourse.bass_utils import run_bass_kernel_spmd

f32 = mybir.dt.float32
f32r = mybir.dt.float32r
bf16 = mybir.dt.bfloat16
AF = mybir.ActivationFunctionType
ALU = mybir.AluOpType

NC = 8          # cores
B, S, D, I = 2, 2048, 1024, 4096
H, HD = 16, 64
T = B * S       # 4096 tokens
DC = D // 128   # 8 feature chunks
NQKV = 384      # qkv cols per core (2 heads x (q,k,v))
IC = 512 // 128  # 4 intermediate chunks per core (I/NC = 512)
EPS = 1e-12
USE_F32R = True

_BF = ml_dtypes.bfloat16


def _bf16(a):
    return np.ascontiguousarray(a.astype(_BF))


def _build(use_f32r=USE_F32R):
    nc = bacc.Bacc("TRN2", target_bir_lowering=False, debug=False, num_devices=NC)

    inp = {}
    def din(name, shape, dt):
        inp[name] = nc.dram_tensor(name, shape, dt, kind="ExternalInput")
        return inp[name]

    xTbf = din("xTbf", [D, T], bf16)
    xT_own = din("xT_own", [128, T], f32)
    wqkv = din("wqkv", [D, NQKV], bf16)
    ncs_qkv = din("ncs_qkv", [1, NQKV], f32)   # -colsum(wqkv folded)
    ow = din("ow", [128, D], bf16)
    w1 = din("w1", [D, 512], bf16)
    ncs1 = din("ncs1", [1, 512], f32)
    w2 = din("w2", [D, 512], bf16)
    outw = din("outw", [512, D], bf16)

    outT = nc.dram_tensor("outT", [128, T], f32, kind="ExternalOutput")

    rdt = f32r if use_f32r else bf16

    with tile.TileContext(nc) as tc:
        with ExitStack() as ctx:
            ep = ctx.enter_context
            cons = ep(tc.tile_pool(name="cons", bufs=1))
            wp = ep(tc.tile_pool(name="wp", bufs=1))
            qkvp = ep(tc.tile_pool(name="qkvp", bufs=1))
            ctxp = ep(tc.tile_pool(name="ctxp", bufs=1))
            xbfp = ep(tc.tile_pool(name="xbfp", bufs=9))
            workp = ep(tc.tile_pool(name="workp", bufs=10))
            sqp = ep(tc.tile_pool(name="sqp", bufs=3))
            xrp = ep(tc.tile_pool(name="xrp", bufs=3))
            drp = ep(tc.tile_pool(name="drp", bufs=4))
            vaugp = ep(tc.tile_pool(name="vaugp", bufs=17))
            expp = ep(tc.tile_pool(name="expp", bufs=17))
            hp = ep(tc.tile_pool(name="hp", bufs=5))
            itp = ep(tc.tile_pool(name="itp", bufs=5))
            rstdp = ep(tc.tile_pool(name="rstdp", bufs=7))
            rowp = ep(tc.tile_pool(name="rowp", bufs=8))
            rowbp = ep(tc.tile_pool(name="rowbp", bufs=2))
            wfp = ep(tc.tile_pool(name="wfp", bufs=4))
            psp = ep(tc.tile_pool(name="psp", bufs=8, space="PSUM"))
            dram = ep(tc.tile_pool(name="dram", bufs=1, space="DRAM"))

            # ---- constants ----
            ident = cons.tile([128, 64], bf16)
            masks.make_identity(nc, ident[0:64, :])
            masks.make_identity(nc, ident[64:128, :])
            ones_col = cons.tile([128, 1], bf16)
            nc.gpsimd.memset(ones_col[:], 1.0)
            ones_all = cons.tile([128, 64], bf16)   # rows reused at any base
            nc.gpsimd.memset(ones_all[:], 1.0)
            invD_f = cons.tile([1, 128], f32)
            nc.gpsimd.memset(invD_f[:], 1.0 / D)
            invD_row = cons.tile([1, 128], rdt)
            nc.vector.tensor_copy(invD_row[:], invD_f[:])
            eps_col = cons.tile([128, 1], f32)
            nc.gpsimd.memset(eps_col[:], EPS)

            ncsq_f = cons.tile([1, NQKV], f32)
            nc.sync.dma_start(ncsq_f[:], ncs_qkv[:])
            ncsq_row = cons.tile([1, NQKV], rdt)
            nc.vector.tensor_copy(ncsq_row[:], ncsq_f[:])
            ncs1_f = cons.tile([1, 512], f32)
            nc.sync.dma_start(ncs1_f[:], ncs1[:])
            ncs1_row = cons.tile([1, 512], rdt)
            nc.vector.tensor_copy(ncs1_row[:], ncs1_f[:])

            def r_(ap):
                return ap

            # ---- weights to SBUF ----
            wqkv_sb = []
            w1_sb, w2_sb = [], []
            for d in range(DC):
                t = wp.tile([128, NQKV], bf16, tag=f"wqkv{d}")
                nc.sync.dma_start(t[:], wqkv[128 * d:128 * (d + 1), :])
                wqkv_sb.append(t)

            # persistent activations
            qkvT = [qkvp.tile([128, T], bf16, tag=f"qkvT{n}", name=f"qkvT{n}") for n in range(3)]
            ctxT = ctxp.tile([128, T], bf16, tag="ctxT", name="ctxT")

            # DRAM scratch
            ar_in = [dram.tile([D, S], bf16, tag=f"ar_in{b}", name=f"ar_in{b}") for b in range(B)]
            rs_attn = [dram.tile([128, S], bf16, tag=f"rs_attn{b}", name=f"rs_attn{b}") for b in range(B)]
            ag_attn = [dram.tile([D, S], bf16, tag=f"ag_attn{b}", name=f"ag_attn{b}", addr_space="Shared") for b in range(B)]
            rs2_in = [dram.tile([D, S // 2], bf16, tag=f"rs2_in{b}{h}", name=f"rs2_in{b}{h}")
                      for b in range(B) for h in range(2)]
            rs2_out = [dram.tile([128, S // 2], bf16, tag=f"rs2_out{b}{h}", name=f"rs2_out{b}{h}")
                       for b in range(B) for h in range(2)]
            ro_dram = [dram.tile([128, S], f32, tag=f"ro{b}", name=f"ro{b}") for b in range(B)]

            RG = [list(range(NC))]

            def ln_stats(feed_tile_fn, tag):
                """feed_tile_fn(d) -> bf16 [128,512] tile AP for chunk d.
                Returns (rstd_tile [128,512] f32, m_row [1,512] f32)."""
                sum_ps = psp.tile([1, 512], f32, tag="ps")
                ssq_ps = psp.tile([1, 512], f32, tag="ps")
                for d in range(DC):
                    xt = feed_tile_fn(d)
                    sq = sqp.tile([128, 512], bf16, tag="sq")
                    nc.vector.tensor_tensor(sq[:], xt, xt, op=ALU.mult)
                    nc.tensor.matmul(sum_ps[:], ones_col[:], xt,
                                     start=(d == 0), stop=(d == DC - 1))
                    nc.tensor.matmul(ssq_ps[:], ones_col[:], sq[:],
                                     start=(d == 0), stop=(d == DC - 1))
                sum_row = rowp.tile([1, 512], rdt, tag="row")
                ssq_row = rowp.tile([1, 512], rdt, tag="row")
                nc.vector.tensor_copy(sum_row[:], sum_ps[:])
                nc.vector.tensor_copy(ssq_row[:], ssq_ps[:])
                mean_ps = psp.tile([128, 512], f32, tag="ps")
                msq_ps = psp.tile([128, 512], f32, tag="ps")
                nc.tensor.matmul(mean_ps[:], invD_row[:], sum_row[:],
                                 start=True, stop=True)
                nc.tensor.matmul(msq_ps[:], invD_row[:], ssq_row[:],
                                 start=True, stop=True)
                msq = wfp.tile([128, 512], f32, tag="wf")
                nc.scalar.activation(msq[:], mean_ps[:], AF.Square)
                var = wfp.tile([128, 512], f32, tag="wf")
                nc.vector.tensor_tensor(var[:], msq_ps[:], msq[:], op=ALU.subtract)
                std = wfp.tile([128, 512], f32, tag="wf")
                nc.scalar.activation(std[:], var[:], AF.Sqrt, bias=eps_col[:])
                rstd = rstdp.tile([128, 512], f32, tag="rstd")
                nc.vector.reciprocal(rstd[:], std[:])
                m_row = rowp.tile([1, 512], rdt, tag="row")
                nc.vector.tensor_copy(m_row[:], mean_ps[0:1, :])
                return rstd, m_row

            # ================= P1: LN1 + QKV =================
            for tb in range(4):           # blocks of 1024 tokens
                xbf = []
                for d in range(DC):
                    t = xbfp.tile([128, 1024], bf16, tag="xbf")
                    nc.sync.dma_start(t[:], xTbf[128 * d:128 * (d + 1),
                                                 1024 * tb:1024 * (tb + 1)])
                    xbf.append(t)
                blk = []
                for tcc in range(2):      # 512-token chunks
                    sl = slice(512 * tcc, 512 * (tcc + 1))
                    rstd, m_row = ln_stats(lambda d: xbf[d][:, sl], f"p1_{tb}_{tcc}")
                    blk.append((sl, rstd, m_row))
                for n in range(3):
                    for (sl, rstd, m_row) in blk:
                        gsl = slice(1024 * tb + sl.start, 1024 * tb + sl.stop)
                        qps = psp.tile([128, 512], f32, tag="ps")
                        for d in range(DC):
                            nc.tensor.matmul(qps[:],
                                             wqkv_sb[d][:, 128 * n:128 * (n + 1)],
                                             xbf[d][:, sl],
                                             start=(d == 0), stop=False)
                        nc.tensor.matmul(qps[:],
                                         ncsq_row[0:1, 128 * n:128 * (n + 1)],
                                         m_row[:], start=False, stop=True)
                        nc.vector.tensor_tensor(qkvT[n][:, gsl], qps[:], rstd[:],
                                                op=ALU.mult)

            # attn-out projection weights (after P1's x-stream has the bus)
            ow_sb = wp.tile([128, D], bf16, tag="ow")
            nc.sync.dma_start(ow_sb[:], ow[:])

            # ================= P2+P3 attention + ow partials, per sequence =====
            for b in range(B):
                for h in range(2):
                    hb = 64 * h
                    bsl0 = S * b
                    vaug = []
                    for kc in range(S // 128):
                        tp = psp.tile([128, 64], bf16, tag="ps")
                        nc.tensor.transpose(
                            tp[:],
                            qkvT[2][hb:hb + 64,
                                    bsl0 + 128 * kc:bsl0 + 128 * (kc + 1)],
                            ident[hb:hb + 64, :])
                        va = vaugp.tile([128, 65], bf16, tag="vaug")
                        nc.vector.tensor_copy(va[:, 0:64], tp[:])
                        nc.vector.tensor_copy(va[:, 64:65], ones_col[:])
                        vaug.append(va)
                    for qc in range(S // 512):
                        qsl = qkvT[0][hb:hb + 64,
                                      bsl0 + 512 * qc:bsl0 + 512 * (qc + 1)]
                        exps = []
                        for kc in range(S // 128):
                            sps = psp.tile([128, 512], f32, tag="ps")
                            nc.tensor.matmul(
                                sps[:],
                                qkvT[1][hb:hb + 64,
                                        bsl0 + 128 * kc:bsl0 + 128 * (kc + 1)],
                                qsl, start=True, stop=True)
                            e = expp.tile([128, 512], bf16, tag="exp")
                            nc.scalar.activation(e[:], sps[:], AF.Exp)
                            exps.append(e)
                        cps = psp.tile([65, 512], f32, tag="ps")
                        for kc in range(S // 128):
                            nc.tensor.matmul(cps[:], vaug[kc][:], exps[kc][:],
                                             start=(kc == 0),
                                             stop=(kc == S // 128 - 1))
                        rr = wfp.tile([128, 512], f32, tag="wf")
                        nc.vector.reciprocal(rr[64:65, :], cps[64:65, :])
                        rbf = rowbp.tile([128, 512], bf16, tag="rbf")
                        nc.vector.tensor_copy(rbf[64:65, :], rr[64:65, :])
                        rbps = psp.tile([64, 512], f32, tag="ps")
                        nc.tensor.matmul(rbps[:], ones_all[64:65, :],
                                         rbf[64:65, :], start=True, stop=True)
                        rb_sb = wfp.tile([128, 512], f32, tag="wf")
                        nc.scalar.activation(rb_sb[0:64, :], rbps[:], AF.Copy)
                        cn = drp.tile([64, 512], bf16, tag="cn")
                        nc.vector.tensor_tensor(cn[:], cps[0:64, :],
                                                rb_sb[0:64, :], op=ALU.mult)
                        # cross-partition placement into ctxT rows 64h
                        nc.sync.dma_start(
                            ctxT[hb:hb + 64,
                                 bsl0 + 512 * qc:bsl0 + 512 * (qc + 1)], cn[:])
                # P3: ow partials for this b
                for oc in range(DC):
                    for tcc in range(S // 512):
                        pps = psp.tile([128, 512], f32, tag="ps")
                        nc.tensor.matmul(
                            pps[:], ow_sb[:, 128 * oc:128 * (oc + 1)],
                            ctxT[:, bsl0 + 512 * tcc:bsl0 + 512 * (tcc + 1)],
                            start=True, stop=True)
                        po = drp.tile([128, 512], bf16, tag="po")
                        nc.scalar.activation(po[:], pps[:], AF.Copy)
                        nc.sync.dma_start(
                            ar_in[b][128 * oc:128 * (oc + 1),
                                     512 * tcc:512 * (tcc + 1)], po[:])
                nc.gpsimd.collective_compute(
                    "ReduceScatter", ALU.add, ins=[ar_in[b].opt()],
                    outs=[rs_attn[b].opt()], replica_groups=RG)
                nc.gpsimd.collective_compute(
                    "AllGather", ALU.bypass, ins=[rs_attn[b].opt()],
                    outs=[ag_attn[b].opt()], replica_groups=RG)

            # MLP weights (deferred: not needed until P4)
            for d in range(DC):
                t1 = wp.tile([128, 512], bf16, tag=f"w1_{d}", name=f"w1sb{d}")
                nc.sync.dma_start(t1[:], w1[128 * d:128 * (d + 1), :])
                w1_sb.append(t1)
                t2 = wp.tile([128, 512], bf16, tag=f"w2_{d}", name=f"w2sb{d}")
                nc.sync.dma_start(t2[:], w2[128 * d:128 * (d + 1), :])
                w2_sb.append(t2)
            outw_sb = []
            for ic in range(IC):
                t3 = wp.tile([128, D], bf16, tag=f"outw{ic}", name=f"outwsb{ic}")
                nc.sync.dma_start(t3[:], outw[128 * ic:128 * (ic + 1), :])
                outw_sb.append(t3)

            # ================= P4: MLP per sequence =================
            for b in range(B):
                bsl0 = S * b
                # resid_own stripe: rs_attn + x_own (f32), to DRAM
                for tcc in range(S // 512):
                    rst = workp.tile([128, 512], bf16, tag="ag")
                    nc.sync.dma_start(rst[:],
                                      rs_attn[b][:, 512 * tcc:512 * (tcc + 1)])
                    xo = wfp.tile([128, 512], f32, tag="wf")
                    nc.sync.dma_start(
                        xo[:], xT_own[:, bsl0 + 512 * tcc:bsl0 + 512 * (tcc + 1)])
                    rof = wfp.tile([128, 512], f32, tag="wf")
                    nc.gpsimd.tensor_tensor(rof[:], rst[:], xo[:], op=ALU.add)
                    nc.sync.dma_start(ro_dram[b][:, 512 * tcc:512 * (tcc + 1)],
                                      rof[:])

                # pass 1: LN2 stats (resid = ag_attn + xTbf, bf16)
                stats = []
                for tcc in range(S // 512):
                    def feed(d, _tcc=tcc):
                        ag = workp.tile([128, 512], bf16, tag="ag")
                        nc.sync.dma_start(
                            ag[:], ag_attn[b][128 * d:128 * (d + 1),
                                              512 * _tcc:512 * (_tcc + 1)])
                        xr = xrp.tile([128, 512], bf16, tag="xr")
                        nc.sync.dma_start(
                            xr[:], xTbf[128 * d:128 * (d + 1),
                                        bsl0 + 512 * _tcc:bsl0 + 512 * (_tcc + 1)])
                        rs = sqp.tile([128, 512], bf16, tag="rsd")
                        nc.vector.tensor_tensor(rs[:], ag[:], xr[:], op=ALU.add)
                        return rs[:]
                    stats.append(ln_stats(feed, f"p4_{b}_{tcc}"))

                # pass 2: h1 = gelu(LN2 @ w1), inter = (ag @ w2) * h1, out partial
                for tcc in range(S // 512):
                    rstd2, m2_row = stats[tcc]
                    ag_t, rs_t = [], []
                    for d in range(DC):
                        ag = workp.tile([128, 512], bf16, tag="ag")
                        nc.sync.dma_start(
                            ag[:], ag_attn[b][128 * d:128 * (d + 1),
                                              512 * tcc:512 * (tcc + 1)])
                        xr = xrp.tile([128, 512], bf16, tag="xr")
                        nc.sync.dma_start(
                            xr[:], xTbf[128 * d:128 * (d + 1),
                                        bsl0 + 512 * tcc:bsl0 + 512 * (tcc + 1)])
                        rs = workp.tile([128, 512], bf16, tag="rsd2")
                        nc.vector.tensor_tensor(rs[:], ag[:], xr[:], op=ALU.add)
                        ag_t.append(ag)
                        rs_t.append(rs)
                    h1_t = []
                    for ic in range(IC):
                        h1ps = psp.tile([128, 512], f32, tag="ps")
                        for d in range(DC):
                            nc.tensor.matmul(
                                h1ps[:], w1_sb[d][:, 128 * ic:128 * (ic + 1)],
                                rs_t[d][:], start=(d == 0), stop=False)
                        nc.tensor.matmul(
                            h1ps[:], ncs1_row[0:1, 128 * ic:128 * (ic + 1)],
                            m2_row[:], start=False, stop=True)
                        gi = wfp.tile([128, 512], f32, tag="wf")
                        nc.vector.tensor_tensor(gi[:], h1ps[:], rstd2[:],
                                                op=ALU.mult)
                        h1 = hp.tile([128, 512], bf16, tag="h1")
                        nc.scalar.activation(h1[:], gi[:], AF.Gelu)
                        h1_t.append(h1)
                    it_t = []
                    for ic in range(IC):
                        h2ps = psp.tile([128, 512], f32, tag="ps")
                        for d in range(DC):
                            nc.tensor.matmul(
                                h2ps[:], w2_sb[d][:, 128 * ic:128 * (ic + 1)],
                                ag_t[d][:], start=(d == 0), stop=(d == DC - 1))
                        it = itp.tile([128, 512], bf16, tag="it")
                        nc.vector.tensor_tensor(it[:], h2ps[:], h1_t[ic][:],
                                                op=ALU.mult)
                        it_t.append(it)
                    for oc in range(DC):
                        ops3 = psp.tile([128, 512], f32, tag="ps")
                        for ic in range(IC):
                            nc.tensor.matmul(
                                ops3[:], outw_sb[ic][:, 128 * oc:128 * (oc + 1)],
                                it_t[ic][:], start=(ic == 0), stop=(ic == IC - 1))
                        po2 = drp.tile([128, 512], bf16, tag="po")
                        nc.scalar.activation(po2[:], ops3[:], AF.Copy)
                        nc.sync.dma_start(
                            rs2_in[2 * b + tcc // 2][128 * oc:128 * (oc + 1),
                                      512 * (tcc % 2):512 * (tcc % 2 + 1)], po2[:])
                for hh in range(2):
                    nc.gpsimd.collective_compute(
                        "ReduceScatter", ALU.add, ins=[rs2_in[2 * b + hh].opt()],
                        outs=[rs2_out[2 * b + hh].opt()], replica_groups=RG)

            # ================= P6: final stripe =================
            for b in range(B):
                bsl0 = S * b
                for tcc in range(S // 512):
                    r2 = workp.tile([128, 512], bf16, tag="ag")
                    nc.sync.dma_start(
                        r2[:], rs2_out[2 * b + tcc // 2][:, 512 * (tcc % 2):
                                                         512 * (tcc % 2 + 1)])
                    ro = wfp.tile([128, 512], f32, tag="wf")
                    nc.sync.dma_start(ro[:],
                                      ro_dram[b][:, 512 * tcc:512 * (tcc + 1)])
                    ot = wfp.tile([128, 512], f32, tag="wf")
                    nc.gpsimd.tensor_tensor(ot[:], r2[:], ro[:], op=ALU.add)
                    nc.sync.dma_start(
                        outT[:, bsl0 + 512 * tcc:bsl0 + 512 * (tcc + 1)], ot[:])

    nc.compile()
    return nc


_NC_CACHE = {}


def kernel(**inputs):
    x = np.asarray(inputs["x"], np.float32)
    norm_w = np.asarray(inputs["norm_w"], np.float32)
    norm_b = np.asarray(inputs["norm_b"], np.float32)
    qkvw = np.asarray(inputs["attn_qkvw"], np.float32)
    qkvb = np.asarray(inputs["attn_qkvb"], np.float32)
    attn_ow = np.asarray(inputs["attn_ow"], np.float32)
    attn_ob = np.asarray(inputs["attn_ob"], np.float32)
    attn_nw = np.asarray(inputs["attn_nw"], np.float32)
    attn_nb = np.asarray(inputs["attn_nb"], np.float32)
    inter_w = np.asarray(inputs["inter_w"], np.float32)
    inter_b = np.asarray(inputs["inter_b"], np.float32)
    inter_w1 = np.asarray(inputs["inter_w1"], np.float32)
    output_w = np.asarray(inputs["output_w"], np.float32)
    output_b = np.asarray(inputs["output_b"], np.float32)

    X = x.reshape(T, D)
    XT = np.ascontiguousarray(X.T)          # [D, T]

    # ---- LN folds (host) ----
    wqkv_f = norm_w[:, None] * qkvw          # [D, 3D]
    bqkv_f = qkvb + norm_b @ qkvw
    wqkv_f = wqkv_f.copy()
    wqkv_f[:, :D] /= np.sqrt(HD)             # attention scale into Q
    bqkv_f = bqkv_f.copy()
    bqkv_f[:D] /= np.sqrt(HD)

    w1_f = attn_nw[:, None] * inter_w        # [D, I]
    b1_f = inter_b + attn_nb @ inter_w

    assert not np.any(bqkv_f) and not np.any(attn_ob) and not np.any(b1_f) \
        and not np.any(output_b), "nonzero biases not wired in this build"

    if ("nc", USE_F32R) not in _NC_CACHE:
        _NC_CACHE[("nc", USE_F32R)] = _build(USE_F32R)
    nc = _NC_CACHE[("nc", USE_F32R)]

    in_maps = []
    for c in range(NC):
        hsl = slice(128 * c, 128 * (c + 1))       # 2 heads' q/k/v cols
        isl = slice(512 * c, 512 * (c + 1))       # intermediate shard
        wq_c = np.concatenate(
            [wqkv_f[:, hsl], wqkv_f[:, D:][:, hsl], wqkv_f[:, 2 * D:][:, hsl]],
            axis=1)                                # [D, 384]
        w1_c = w1_f[:, isl]
        w2_c = inter_w1[:, isl]
        in_maps.append({
            "xTbf": _bf16(XT),
            "xT_own": np.ascontiguousarray(XT[hsl, :]),
            "wqkv": _bf16(wq_c),
            "ncs_qkv": np.ascontiguousarray(-wq_c.sum(0, keepdims=True)),
            "ow": _bf16(attn_ow[hsl, :]),
            "w1": _bf16(w1_c),
            "ncs1": np.ascontiguousarray(-w1_c.sum(0, keepdims=True)),
            "w2": _bf16(w2_c),
            "outw": _bf16(output_w[isl, :]),
        })

    global _LAST_IN_MAPS
    _LAST_IN_MAPS = in_maps
    res = run_bass_kernel_spmd(nc, in_maps, list(range(NC)))
    outT = np.concatenate([res.results[c]["outT"] for c in range(NC)], axis=0)
    return np.ascontiguousarray(outT.T).reshape(B, S, D).astype(np.float32)


if __name__ == "__main__":
    pass



# revision 29
# speedup vs baseline: 1.3525x; 1.3525x over previous
"""Tensor-parallel DeepSpeed encoder-decoder block on 8 TRN2 NeuronCores.

Sharding (mp_group scheme): attn_qkvw / inter_w / inter_w1 column-sharded
(2 heads / 512 intermediate cols per core), attn_ow / output_w row-sharded.
Post-attn all-reduce = ReduceScatter + AllGather (fp8 payloads, x64 scale
folded host-side); post-output_w all-reduce = ReduceScatter only (fp8, x256
scale), each core finishing its own 128-row feature stripe.

Device compute: fp8(e4m3) DoubleRow matmuls (K=256 per instruction) for the
QKV / MLP / probs-V GEMMs; bf16 for scores and attn-out. LayerNorms fold
gamma/beta into weights host-side; mean correction is a rank-1 matmul; the
1/std factor is exp(-0.5*ln(var)) on the Scalar engine (stays in the
natural_log_exp table set) broadcast via a K=1 matmul and applied at PSUM
drain. LN2 stats ride inside the AllGather as 2 extra rows per rank
(per-stripe sum / sum-of-squares of the residual), so no extra collective
and no second pass over the activations. Softmax runs in transposed layout
with the denominator produced by a ones-column augmentation of V; the
reciprocal is exp(-ln(d)+ln(32)) on Scalar (no slow DVE reciprocal).
GpSimd is reserved for collective triggers so compute queues never block on
collective waits.
"""
from contextlib import ExitStack

import math
import numpy as np
import ml_dtypes

import concourse.bacc as bacc
import concourse.mybir as mybir
import concourse.tile as tile
from concourse import masks
from concourse.bass_utils import run_bass_kernel_spmd

f32 = mybir.dt.float32
bf16 = mybir.dt.bfloat16
fp8 = mybir.dt.float8e4
AF = mybir.ActivationFunctionType
ALU = mybir.AluOpType
DR = mybir.MatmulPerfMode.DoubleRow

NC = 8
B, S, D, I = 2, 2048, 1024, 4096
H, HD = 16, 64
T = B * S
DC = D // 128          # 8 feature chunks
NQKV = 384             # qkv cols per core
EPS = 1e-12
LN32 = math.log(32.0)

SIM_GELU = False       # sim doesn't implement Gelu; swap for Sigmoid there

SW = 16.0              # fp8 weight scale (wqkv, w1, w2, outw)
SCTX = 32.0            # ctxT scale
SAR = 64.0             # attn partial / RS / AG scale
SO2 = 256.0            # mlp partial / RS2 scale

_BF = ml_dtypes.bfloat16
_F8 = ml_dtypes.float8_e4m3fn


def _bf(a):
    return np.ascontiguousarray(np.asarray(a, np.float32).astype(_BF))


def _f8(a):
    return np.ascontiguousarray(
        np.clip(np.asarray(a, np.float32), -240.0, 240.0).astype(_F8))


def _planes(w):  # [D, F] -> [128, D//128, F]
    w = np.asarray(w, np.float32)
    return w.reshape(DC, 128, w.shape[1]).transpose(1, 0, 2)


def _build():
    nc = bacc.Bacc("TRN2", target_bir_lowering=False, debug=False,
                   num_devices=NC)

    inp = {}

    def din(name, shape, dt):
        inp[name] = nc.dram_tensor(name, shape, dt, kind="ExternalInput")
        return inp[name]

    xp = din("xp", [128, DC, T], fp8)          # x feature-planes
    x_own = din("x_own", [128, T], f32)        # core's 128-feat stripe of x
    wqkv = din("wqkv", [128, DC, NQKV], fp8)   # 16x folded-LN1 qkv weights
    ncsq = din("ncsq", [1, NQKV], f32)         # -colsum(wqkv_dev)/D
    ow = din("ow", [128, D], bf16)             # 2x attn_ow rows
    w1 = din("w1", [128, DC, 512], fp8)        # 16x folded-LN2 inter_w
    ncs1 = din("ncs1", [1, 512], f32)          # -colsum(w1_dev)*64/D
    w2 = din("w2", [128, DC, 512], fp8)        # inter_w1 / 4
    outw = din("outw", [128, 4, D], fp8)       # 8x output_w
    seli = din("seli", [16, 2], fp8)           # LN2 stats combiner

    outT = nc.dram_tensor("outT", [128, T], f32, kind="ExternalOutput")

    RG = [list(range(NC))]

    with tile.TileContext(nc) as tc:
        with ExitStack() as ctx:
            ep = ctx.enter_context
            cons = ep(tc.tile_pool(name="cons", bufs=1))
            wp = ep(tc.tile_pool(name="wp", bufs=1))
            qkvp = ep(tc.tile_pool(name="qkvp", bufs=1))
            rowp = ep(tc.tile_pool(name="rowp", bufs=6))
            sqp = ep(tc.tile_pool(name="sqp", bufs=3))
            vaugp = ep(tc.tile_pool(name="vaugp", bufs=18))
            expp = ep(tc.tile_pool(name="expp", bufs=4))
            drp = ep(tc.tile_pool(name="drp", bufs=6))
            agp = ep(tc.tile_pool(name="agp", bufs=6))
            rsdp = ep(tc.tile_pool(name="rsdp", bufs=6))
            gip = ep(tc.tile_pool(name="gip", bufs=3))
            h1p = ep(tc.tile_pool(name="h1p", bufs=3))
            itp = ep(tc.tile_pool(name="itp", bufs=4))
            wfp = ep(tc.tile_pool(name="wfp", bufs=4))
            # PSUM: acc(2) + bc(1) + sc(1x4banks) + tp(1) = 8 banks
            psA = ep(tc.tile_pool(name="psA", bufs=2, space="PSUM"))
            psB = ep(tc.tile_pool(name="psB", bufs=1, space="PSUM"))
            psS = ep(tc.tile_pool(name="psS", bufs=1, space="PSUM"))
            psT = ep(tc.tile_pool(name="psT", bufs=1, space="PSUM"))
            dram = ep(tc.tile_pool(name="dram", bufs=1, space="DRAM"))

            # ---------------- constants ----------------
            ident = cons.tile([128, 64], bf16)
            masks.make_identity(nc, ident[0:64, :])
            masks.make_identity(nc, ident[64:128, :])
            ones_col = cons.tile([128, 1], bf16)
            nc.vector.memset(ones_col[:], 1.0)
            onesp8 = cons.tile([128, 2, 128], fp8)  # DR colsum lhsT (all ones)
            nc.vector.memset(onesp8[:], 1.0)
            inv16_col = cons.tile([1, 128], bf16)   # bcast lhsT: 1/SW
            nc.vector.memset(inv16_col[:], 1.0 / SW)
            ones64_row = cons.tile([1, 64], bf16)
            nc.vector.memset(ones64_row[:], 1.0)
            eps_col = cons.tile([128, 1], f32)
            nc.vector.memset(eps_col[:], EPS)
            ln32_col = cons.tile([128, 1], f32)
            nc.vector.memset(ln32_col[:], LN32)
            sel = cons.tile([16, 2], fp8)           # stats combiner
            nc.sync.dma_start(sel[:], seli[:])

            ncsq_f = cons.tile([1, NQKV], f32)
            nc.sync.dma_start(ncsq_f[:], ncsq[:])
            ncsq_row = cons.tile([1, NQKV], bf16)
            nc.vector.tensor_copy(ncsq_row[:], ncsq_f[:])
            ncs1_f = cons.tile([1, 512], f32)
            nc.sync.dma_start(ncs1_f[:], ncs1[:])
            ncs1_row = cons.tile([1, 512], bf16)
            nc.vector.tensor_copy(ncs1_row[:], ncs1_f[:])

            # ---------------- persistent SBUF ----------------
            xsb = wp.tile([128, DC, T], fp8, name="xsb")
            for q in range(4):
                nc.sync.dma_start(xsb[:, :, 1024 * q:1024 * (q + 1)],
                                  xp[:, :, 1024 * q:1024 * (q + 1)])
            xo_sb = wp.tile([128, T], f32, name="xo_sb")
            nc.sync.dma_start(xo_sb[:], x_own[:])
            wqkv_sb = wp.tile([128, DC, NQKV], fp8, name="wqkv_sb")
            nc.sync.dma_start(wqkv_sb[:], wqkv[:])
            ow_sb = wp.tile([128, D], bf16, name="ow_sb")
            nc.sync.dma_start(ow_sb[:], ow[:])

            qkvT = [qkvp.tile([128, T], bf16, name=f"qkvT{n}") for n in range(3)]
            ctxT = qkvp.tile([128, T], bf16, name="ctxT")

            # ---------------- DRAM scratch ----------------
            ar_in = [dram.tile([D, S], fp8, name=f"ar_in{b}") for b in range(B)]
            ag_in = [dram.tile([130, S], fp8, name=f"ag_in{b}") for b in range(B)]
            ag_out = [dram.tile([NC, 130, S], fp8, name=f"ag_out{b}",
                                addr_space="Shared") for b in range(B)]
            rs2_in = [dram.tile([D, S // 2], fp8, name=f"rs2_in{b}{h}")
                      for b in range(B) for h in range(2)]
            rs2_out = [dram.tile([128, S // 2], fp8, name=f"rs2_out{b}{h}")
                       for b in range(B) for h in range(2)]

            drain_rr = [0]

            def drain(dst, src):
                """PSUM -> SBUF drain, alternating vector/scalar engines."""
                drain_rr[0] ^= 1
                if drain_rr[0]:
                    nc.vector.tensor_copy(dst, src)
                else:
                    nc.scalar.activation(dst, src, AF.Copy)

            # =========== P1: LN1 stats + QKV (pipelined) ===========
            def p1_stats(blk):
                """LN1 stats for 512-token block -> (rstd_bc producer, m_row)."""
                tsl = slice(512 * blk, 512 * (blk + 1))
                sq = [sqp.tile([128, 2, 512], fp8, tag="sq", name="sq")
                      for _ in range(4)]
                for p in range(4):
                    nc.vector.tensor_tensor(sq[p][:], xsb[:, 2 * p:2 * p + 2, tsl],
                                            xsb[:, 2 * p:2 * p + 2, tsl],
                                            op=ALU.mult)
                srow = psA.tile([128, 512], f32, tag="acc", name="srow")
                qrow = psA.tile([128, 512], f32, tag="acc", name="qrow")
                for p in range(4):
                    nc.tensor.matmul(srow[:], onesp8[:],
                                     xsb[:, 2 * p:2 * p + 2, tsl],
                                     start=(p == 0), stop=(p == 3), perf_mode=DR)
                for p in range(4):
                    nc.tensor.matmul(qrow[:], onesp8[:], sq[p][:],
                                     start=(p == 0), stop=(p == 3), perf_mode=DR)
                msq = rowp.tile([1, 512], f32, tag="row", name="msq")
                nc.scalar.activation(msq[:], srow[0:1, :], AF.Square, scale=1.0 / D)
                vv = rowp.tile([1, 512], f32, tag="row", name="vv")
                nc.vector.scalar_tensor_tensor(vv[:], qrow[0:1, :], 1.0 / D,
                                               msq[:], op0=ALU.mult,
                                               op1=ALU.subtract)
                lg = rowp.tile([1, 512], f32, tag="row", name="lg")
                nc.scalar.activation(lg[:], vv[:], AF.Ln, bias=eps_col[0:1, :])
                rstd_row = rowp.tile([1, 512], bf16, tag="rowb", name="rstd_row")
                nc.scalar.activation(rstd_row[:], lg[:], AF.Exp, scale=-0.5)
                m_row = rowp.tile([1, 512], bf16, tag="rowb", name="m_row")
                nc.vector.tensor_copy(m_row[:], srow[0:1, :])
                return rstd_row, m_row

            def p1_qkv(blk, rstd_row, m_row):
                tsl = slice(512 * blk, 512 * (blk + 1))
                rps = psB.tile([128, 512], f32, tag="bc", name="rps")
                nc.tensor.matmul(rps[:], inv16_col[:], rstd_row[:],
                                 start=True, stop=True)
                rbc = gip.tile([128, 512], f32, tag="bcs", name="rbc")
                nc.scalar.activation(rbc[:], rps[:], AF.Copy)
                for n in range(3):
                    qps = psA.tile([128, 512], f32, tag="acc", name="qps")
                    for p in range(4):
                        nc.tensor.matmul(
                            qps[:], wqkv_sb[:, 2 * p:2 * p + 2,
                                            128 * n:128 * (n + 1)],
                            xsb[:, 2 * p:2 * p + 2, tsl],
                            start=(p == 0), stop=False, perf_mode=DR)
                    nc.tensor.matmul(qps[:],
                                     ncsq_row[0:1, 128 * n:128 * (n + 1)],
                                     m_row[:], start=False, stop=True)
                    nc.vector.tensor_tensor(qkvT[n][:, tsl], qps[:], rbc[:],
                                            op=ALU.mult)

            # =========== P2 attention pieces ===========
            def attn_vaug(b, h):
                hb = 64 * h
                bsl0 = S * b
                vaug = []
                for kp in range(8):           # kc pairs
                    va = vaugp.tile([128, 2, 128], fp8, tag="vaug", name="va")
                    nc.vector.memset(va[:, :, 64:65], 1.0)
                    nc.vector.memset(va[:, :, 65:128], 0.0)
                    for j in range(2):
                        kc = 2 * kp + j
                        tp = psT.tile([128, 64], bf16, tag="tp", name="tp")
                        nc.tensor.transpose(
                            tp[:], qkvT[2][hb:hb + 64,
                                           bsl0 + 128 * kc:bsl0 + 128 * (kc + 1)],
                            ident[hb:hb + 64, :])
                        nc.vector.tensor_copy(va[:, j, 0:64], tp[:])
                    vaug.append(va)
                return vaug

            def attn_qc(b, h, qc, vaug):
                hb = 64 * h
                bsl0 = S * b
                qsl = qkvT[0][hb:hb + 64, bsl0 + 512 * qc:bsl0 + 512 * (qc + 1)]
                exps = []
                for r in range(4):            # rounds of 4 kc
                    sc = psS.tile([128, 4, 512], f32, tag="sc", name="sc")
                    for j in range(4):
                        kc = 4 * r + j
                        nc.tensor.matmul(
                            sc[:, j, :],
                            qkvT[1][hb:hb + 64,
                                    bsl0 + 128 * kc:bsl0 + 128 * (kc + 1)],
                            qsl, start=True, stop=True)
                    e = expp.tile([128, 4, 512], fp8, tag="exp", name="e")
                    nc.scalar.activation(e[:], sc[:], AF.Exp)
                    exps.append(e)
                cps = psA.tile([128, 512], f32, tag="acc", name="cps")
                for r in range(4):
                    for j in range(2):
                        nc.tensor.matmul(cps[:], vaug[2 * r + j][:],
                                         exps[r][:, 2 * j:2 * j + 2, :],
                                         start=(r == 0 and j == 0),
                                         stop=(r == 3 and j == 1), perf_mode=DR)
                ld = rowp.tile([1, 512], f32, tag="row", name="ld")
                nc.scalar.activation(ld[:], cps[64:65, :], AF.Ln)
                rr = rowp.tile([1, 512], bf16, tag="rowb", name="rr")
                nc.scalar.activation(rr[:], ld[:], AF.Exp, scale=-1.0,
                                     bias=ln32_col[0:1, :])
                rb = psB.tile([128, 512], f32, tag="bc", name="rb")
                nc.tensor.matmul(rb[0:64, :], ones64_row[:], rr[:],
                                 start=True, stop=True)
                rbs = gip.tile([128, 512], f32, tag="bcs", name="rbs")
                nc.scalar.activation(rbs[0:64, :], rb[0:64, :], AF.Copy)
                cn = drp.tile([64, 512], bf16, tag="cn", name="cn")
                nc.vector.tensor_tensor(cn[:], cps[0:64, :], rbs[0:64, :],
                                        op=ALU.mult)
                nc.sync.dma_start(
                    ctxT[hb:hb + 64, bsl0 + 512 * qc:bsl0 + 512 * (qc + 1)],
                    cn[:])

            # =========== P3: attn-out partials ===========
            def p3(b):
                bsl0 = S * b
                for tcc in range(4):
                    for oc in range(DC):
                        ops3 = psA.tile([128, 512], f32, tag="acc", name="ops3")
                        nc.tensor.matmul(
                            ops3[:], ow_sb[:, 128 * oc:128 * (oc + 1)],
                            ctxT[:, bsl0 + 512 * tcc:bsl0 + 512 * (tcc + 1)],
                            start=True, stop=True)
                        po = drp.tile([128, 512], fp8, tag="po", name="po")
                        drain(po[:], ops3[:])
                        nc.sync.dma_start(
                            ar_in[b][128 * oc:128 * (oc + 1),
                                     512 * tcc:512 * (tcc + 1)], po[:])

            # =========== stripe stats -> AG payload ===========
            def stripe_stats(b):
                for tcc in range(4):
                    tsl = slice(512 * tcc, 512 * (tcc + 1))
                    gsl = slice(S * b + 512 * tcc, S * b + 512 * (tcc + 1))
                    rs_t = wfp.tile([128, 512], fp8, tag="rs8", name="rs_t")
                    nc.sync.dma_start(rs_t[:], ag_in[b][0:128, tsl])
                    rof = wfp.tile([128, 512], f32, tag="wf", name="rof")
                    nc.vector.scalar_tensor_tensor(rof[:], rs_t[:], 1.0 / SAR,
                                                   xo_sb[:, gsl],
                                                   op0=ALU.mult, op1=ALU.add)
                    rof_bf = wfp.tile([128, 512], bf16, tag="wfb", name="rof_bf")
                    nc.vector.tensor_copy(rof_bf[:], rof[:])
                    sqf = wfp.tile([128, 512], bf16, tag="wfb", name="sqf")
                    nc.vector.tensor_tensor(sqf[:], rof[:], rof[:], op=ALU.mult)
                    srow = psA.tile([128, 512], f32, tag="acc", name="ssrow")
                    nc.tensor.matmul(srow[0:1, :], ones_col[:], rof_bf[:],
                                     start=True, stop=True)
                    qrow = psA.tile([128, 512], f32, tag="acc", name="sqrow")
                    nc.tensor.matmul(qrow[0:1, :], ones_col[:], sqf[:],
                                     start=True, stop=True)
                    st8a = rowp.tile([1, 512], fp8, tag="st8", name="st8a")
                    nc.vector.tensor_scalar(st8a[:], srow[0:1, :],
                                            1.0 / SAR, None, op0=ALU.mult)
                    st8b = rowp.tile([1, 512], fp8, tag="st8", name="st8b")
                    nc.vector.tensor_scalar(st8b[:], qrow[0:1, :],
                                            1.0 / SAR, None, op0=ALU.mult)
                    nc.sync.dma_start(ag_in[b][128:129, tsl], st8a[:])
                    nc.sync.dma_start(ag_in[b][129:130, tsl], st8b[:])

            # =========== P4: MLP block (one 512-token chunk) ===========
            def p4(b, tcc):
                tsl = slice(512 * tcc, 512 * (tcc + 1))
                gsl = slice(S * b + 512 * tcc, S * b + 512 * (tcc + 1))
                # LN2 stats from AG rows
                st = sqp.tile([16, 512], fp8, tag="st", name="st")
                nc.sync.dma_start(st[:], ag_out[b][0:NC, 128:130, tsl])
                tot = psA.tile([128, 512], f32, tag="acc", name="tot")
                nc.tensor.matmul(tot[0:1, :], sel[:, 0:1], st[:],
                                 start=True, stop=True)
                totq = psA.tile([128, 512], f32, tag="acc", name="totq")
                nc.tensor.matmul(totq[0:1, :], sel[:, 1:2], st[:],
                                 start=True, stop=True)
                msq = rowp.tile([1, 512], f32, tag="row", name="msq2")
                nc.scalar.activation(msq[:], tot[0:1, :], AF.Square, scale=1.0 / SW)
                vv = rowp.tile([1, 512], f32, tag="row", name="vv2")
                nc.vector.scalar_tensor_tensor(vv[:], totq[0:1, :], 1.0 / SW,
                                               msq[:], op0=ALU.mult,
                                               op1=ALU.subtract)
                lg = rowp.tile([1, 512], f32, tag="row", name="lg2")
                nc.scalar.activation(lg[:], vv[:], AF.Ln, bias=eps_col[0:1, :])
                rstd2 = rowp.tile([1, 512], bf16, tag="rowb", name="rstd2")
                nc.scalar.activation(rstd2[:], lg[:], AF.Exp, scale=-0.5)
                m2 = rowp.tile([1, 512], bf16, tag="rowb", name="m2")
                nc.vector.tensor_copy(m2[:], tot[0:1, :])

                # ag planes + resid
                ag_t, rs_t = [], []
                for p in range(4):
                    ag = agp.tile([128, 2, 512], fp8, tag="ag", name="ag")
                    for j in range(2):
                        d = 2 * p + j
                        nc.sync.dma_start(ag[:, j, :],
                                          ag_out[b][d:d + 1, 0:128, tsl])
                    rsd = rsdp.tile([128, 2, 512], fp8, tag="rsd", name="rsd")
                    nc.vector.scalar_tensor_tensor(
                        rsd[:], ag[:], 1.0 / SAR, xsb[:, 2 * p:2 * p + 2, gsl],
                        op0=ALU.mult, op1=ALU.add)
                    ag_t.append(ag)
                    rs_t.append(rsd)

                rps = psB.tile([128, 512], f32, tag="bc", name="rps2")
                nc.tensor.matmul(rps[:], inv16_col[:], rstd2[:],
                                 start=True, stop=True)
                rbc = gip.tile([128, 512], f32, tag="bcs", name="rbc2")
                nc.scalar.activation(rbc[:], rps[:], AF.Copy)

                # h2 then h1/gelu/it, then out partials
                it_t = []
                for icp in range(2):
                    it = itp.tile([128, 2, 512], fp8, tag="it", name="it")
                    it_t.append(it)
                for ic in range(4):
                    h2ps = psA.tile([128, 512], f32, tag="acc", name="h2ps")
                    for p in range(4):
                        nc.tensor.matmul(
                            h2ps[:], w2_sb[:, 2 * p:2 * p + 2,
                                           128 * ic:128 * (ic + 1)],
                            ag_t[p][:], start=(p == 0), stop=(p == 3),
                            perf_mode=DR)
                    h1ps = psA.tile([128, 512], f32, tag="acc", name="h1ps")
                    for p in range(4):
                        nc.tensor.matmul(
                            h1ps[:], w1_sb[:, 2 * p:2 * p + 2,
                                           128 * ic:128 * (ic + 1)],
                            rs_t[p][:], start=(p == 0), stop=False,
                            perf_mode=DR)
                    nc.tensor.matmul(h1ps[:],
                                     ncs1_row[0:1, 128 * ic:128 * (ic + 1)],
                                     m2[:], start=False, stop=True)
                    gi = gip.tile([128, 512], f32, tag="gi", name="gi")
                    nc.vector.tensor_tensor(gi[:], h1ps[:], rbc[:], op=ALU.mult)
                    h1 = h1p.tile([128, 512], bf16, tag="h1", name="h1")
                    nc.scalar.activation(h1[:], gi[:],
                                         AF.Sigmoid if SIM_GELU else AF.Gelu)
                    nc.vector.scalar_tensor_tensor(
                        it_t[ic // 2][:, ic % 2, :], h2ps[:], 1.0 / SW, h1[:],
                        op0=ALU.mult, op1=ALU.mult)
                for oc in range(DC):
                    ops3 = psA.tile([128, 512], f32, tag="acc", name="ops4")
                    for icp in range(2):
                        nc.tensor.matmul(
                            ops3[:], outw_sb[:, 2 * icp:2 * icp + 2,
                                             128 * oc:128 * (oc + 1)],
                            it_t[icp][:], start=(icp == 0), stop=(icp == 1),
                            perf_mode=DR)
                    po2 = drp.tile([128, 512], fp8, tag="po", name="po2")
                    drain(po2[:], ops3[:])
                    nc.sync.dma_start(
                        rs2_in[2 * b + tcc // 2][128 * oc:128 * (oc + 1),
                                                 512 * (tcc % 2):
                                                 512 * (tcc % 2 + 1)], po2[:])

            # =========== P6: final stripe chunks ===========
            def p6(b, tcc):
                tsl = slice(512 * tcc, 512 * (tcc + 1))
                gsl = slice(S * b + 512 * tcc, S * b + 512 * (tcc + 1))
                rs_t = wfp.tile([128, 512], fp8, tag="rs8", name="rs_t6")
                nc.sync.dma_start(rs_t[:], ag_in[b][0:128, tsl])
                t1 = wfp.tile([128, 512], f32, tag="wf", name="t1")
                nc.vector.scalar_tensor_tensor(t1[:], rs_t[:], 1.0 / SAR,
                                               xo_sb[:, gsl],
                                               op0=ALU.mult, op1=ALU.add)
                r2 = wfp.tile([128, 512], fp8, tag="rs8", name="r2")
                nc.sync.dma_start(
                    r2[:], rs2_out[2 * b + tcc // 2][:, 512 * (tcc % 2):
                                                     512 * (tcc % 2 + 1)])
                ot = wfp.tile([128, 512], f32, tag="wf", name="ot")
                nc.vector.scalar_tensor_tensor(ot[:], r2[:], 1.0 / SO2, t1[:],
                                               op0=ALU.mult, op1=ALU.add)
                nc.sync.dma_start(outT[:, gsl], ot[:])

            # ================= EMISSION =================
            # P1 blocks 0..3 (batch 0 tokens), stats one block ahead
            st0 = p1_stats(0)
            stats_next = p1_stats(1)
            p1_qkv(0, *st0)
            for blk in range(1, 4):
                cur = stats_next
                stats_next = p1_stats(blk + 1)
                p1_qkv(blk, *cur)

            # MLP weights now (DMA overlaps attention)
            w1_sb = wp.tile([128, DC, 512], fp8, name="w1_sb")
            nc.sync.dma_start(w1_sb[:], w1[:])
            w2_sb = wp.tile([128, DC, 512], fp8, name="w2_sb")
            nc.sync.dma_start(w2_sb[:], w2[:])
            outw_sb = wp.tile([128, 4, D], fp8, name="outw_sb")
            nc.sync.dma_start(outw_sb[:], outw[:])

            # attention(b0) interleaved with P1 blocks 4..7 (batch 1 tokens)
            rem = [4, 5, 6, 7]
            for h in range(2):
                vaug = attn_vaug(0, h)
                for qc in range(4):
                    if rem:
                        blk = rem.pop(0)
                        cur = stats_next
                        stats_next = p1_stats(blk + 1) if blk < 7 else None
                        p1_qkv(blk, *cur)
                    attn_qc(0, h, qc, vaug)

            p3(0)
            nc.gpsimd.collective_compute(
                "ReduceScatter", ALU.add, ins=[ar_in[0].opt()],
                outs=[ag_in[0][0:128, :].opt()], replica_groups=RG)

            # attention(b1) h0 covers RS(b0) latency
            vaug = attn_vaug(1, 0)
            for qc in range(4):
                attn_qc(1, 0, qc, vaug)

            stripe_stats(0)
            nc.gpsimd.collective_compute(
                "AllGather", ALU.bypass, ins=[ag_in[0].opt()],
                outs=[ag_out[0].opt()], replica_groups=RG)

            vaug = attn_vaug(1, 1)
            for qc in range(4):
                attn_qc(1, 1, qc, vaug)

            p3(1)
            nc.gpsimd.collective_compute(
                "ReduceScatter", ALU.add, ins=[ar_in[1].opt()],
                outs=[ag_in[1][0:128, :].opt()], replica_groups=RG)

            # P4(b0) first half covers RS(b1)
            p4(0, 0)
            p4(0, 1)
            stripe_stats(1)
            nc.gpsimd.collective_compute(
                "AllGather", ALU.bypass, ins=[ag_in[1].opt()],
                outs=[ag_out[1].opt()], replica_groups=RG)
            nc.gpsimd.collective_compute(
                "ReduceScatter", ALU.add, ins=[rs2_in[0].opt()],
                outs=[rs2_out[0].opt()], replica_groups=RG)
            p4(0, 2)
            p4(0, 3)
            nc.gpsimd.collective_compute(
                "ReduceScatter", ALU.add, ins=[rs2_in[1].opt()],
                outs=[rs2_out[1].opt()], replica_groups=RG)
            p4(1, 0)
            p4(1, 1)
            nc.gpsimd.collective_compute(
                "ReduceScatter", ALU.add, ins=[rs2_in[2].opt()],
                outs=[rs2_out[2].opt()], replica_groups=RG)
            p6(0, 0)
            p6(0, 1)
            p4(1, 2)
            p4(1, 3)
            nc.gpsimd.collective_compute(
                "ReduceScatter", ALU.add, ins=[rs2_in[3].opt()],
                outs=[rs2_out[3].opt()], replica_groups=RG)
            p6(0, 2)
            p6(0, 3)
            for tcc in range(4):
                p6(1, tcc)

    nc.compile()
    return nc


_NC_CACHE = {}


def make_in_maps(**inputs):
    x = np.asarray(inputs["x"], np.float32)
    norm_w = np.asarray(inputs["norm_w"], np.float32)
    norm_b = np.asarray(inputs["norm_b"], np.float32)
    qkvw = np.asarray(inputs["attn_qkvw"], np.float32)
    qkvb = np.asarray(inputs["attn_qkvb"], np.float32)
    attn_ow = np.asarray(inputs["attn_ow"], np.float32)
    attn_ob = np.asarray(inputs["attn_ob"], np.float32)
    attn_nw = np.asarray(inputs["attn_nw"], np.float32)
    attn_nb = np.asarray(inputs["attn_nb"], np.float32)
    inter_w = np.asarray(inputs["inter_w"], np.float32)
    inter_b = np.asarray(inputs["inter_b"], np.float32)
    inter_w1 = np.asarray(inputs["inter_w1"], np.float32)
    output_w = np.asarray(inputs["output_w"], np.float32)
    output_b = np.asarray(inputs["output_b"], np.float32)

    X = x.reshape(T, D)
    XT = np.ascontiguousarray(X.T)              # [D, T]

    # LN folds
    wqkv_f = norm_w[:, None] * qkvw
    bqkv_f = qkvb + norm_b @ qkvw
    wqkv_f = wqkv_f.copy()
    wqkv_f[:, :D] /= np.sqrt(HD)
    w1_f = attn_nw[:, None] * inter_w
    b1_f = inter_b + attn_nb @ inter_w

    assert not np.any(bqkv_f) and not np.any(attn_ob) and not np.any(b1_f) \
        and not np.any(output_b), "nonzero biases not wired in this build"

    xp_all = _f8(_planes(XT))                   # x planes [128, 8, T]

    in_maps = []
    for c in range(NC):
        hsl = slice(128 * c, 128 * (c + 1))
        isl = slice(512 * c, 512 * (c + 1))
        wq_c = np.concatenate(
            [wqkv_f[:, hsl], wqkv_f[:, D:][:, hsl], wqkv_f[:, 2 * D:][:, hsl]],
            axis=1)                             # [D, 384]
        wq8 = _f8(_planes(wq_c * SW))
        w1_c = w1_f[:, isl]
        w18 = _f8(_planes(w1_c * SW))
        w28 = _f8(_planes(inter_w1[:, isl] * (SCTX * SW / SAR)))
        ou8 = _f8(output_w[isl, :].reshape(4, 128, D).transpose(1, 0, 2)
                  * (SO2 / SCTX))
        wq_deq = wq8.astype(np.float32).transpose(1, 0, 2).reshape(D, NQKV)
        w1_deq = w18.astype(np.float32).transpose(1, 0, 2).reshape(D, 512)
        in_maps.append({
            "xp": xp_all,
            "x_own": np.ascontiguousarray(XT[hsl, :]),
            "wqkv": wq8,
            "ncsq": np.ascontiguousarray(-wq_deq.sum(0, keepdims=True) / D),
            "ow": _bf(attn_ow[hsl, :] * (SAR / SCTX)),
            "w1": w18,
            "ncs1": np.ascontiguousarray(
                -w1_deq.sum(0, keepdims=True) / SW),
            "w2": w28,
            "outw": np.ascontiguousarray(ou8),
            "seli": _f8(np.tile(np.eye(2, dtype=np.float32), (8, 1))),
        })
    return in_maps


def kernel(**inputs):
    if "nc" not in _NC_CACHE:
        _NC_CACHE["nc"] = _build()
    nc = _NC_CACHE["nc"]
    in_maps = make_in_maps(**inputs)
    global _LAST_IN_MAPS
    _LAST_IN_MAPS = in_maps
    res = run_bass_kernel_spmd(nc, in_maps, list(range(NC)))
    outT = np.concatenate([res.results[c]["outT"] for c in range(NC)], axis=0)
    return np.ascontiguousarray(outT.T).reshape(B, S, D).astype(np.float32)


if __name__ == "__main__":
    pass
